# revision 13
# baseline (speedup 1.0000x reference)
"""Trainium2 Bass kernel for nn_GaussianMaskRenderer.

Strategy
--------
The reference splats N=6144 depth-sorted gaussians (5x5 footprint, radius
always 2) into 224x448 framebuffers for 6 render lanes (b*t*view) x 3 alpha
branches (all/dyn/sta) with sequential front-to-back compositing.

The sequential per-pixel recurrence has the closed form
    T_i = max(T_{i-1} * (1 - la_i), 0.001),   contrib_i = la_i * T_{i-1}
i.e. a mult/max prefix scan over each pixel's depth-ordered splat list -- an
exact match for the VectorEngine's tensor_tensor_scan(op0=mult, op1=max).

Host side (pure indexing + the reference's trivial per-gaussian projection):
build per-pixel depth-ordered (gaussian,offset) pair lists, group pixels into
occupancy classes, and pack every pair's (du, dv, alpha_all/dyn/sta, rgb)
into dense [128, F] planes per core, with a boundary slot (d0=0, d1=1) in
front of every pixel run so one chained scan per branch resets per pixel.

Device side (8 NeuronCores, data-parallel over pixel work): exp/compositing/
reductions as dense vector ops; per-class 3D-AP reduces produce per-pixel
rgb/alpha; GPSIMD partition-reduce produces the sm/tr scalars.
"""

import numpy as np

import concourse.bass as bass  # noqa: F401  (AP helpers)
import concourse.mybir as mybir
import concourse.tile as tile
from concourse import bacc
from concourse.bass_utils import run_bass_kernel_spmd

B, T, V, GH, GW = 1, 2, 3, 32, 64
H, W = 224, 448
HW = H * W
R = 2
NLANES = B * T * V           # 6 render lanes
N = V * GH * GW              # 6144 gaussians per (b,t) frame
NCORES = 8
NPART = 128
SM_COLS = N // NPART         # 48 gaussian columns per lane in the sm table
f32 = np.float32

_OFFS = np.array([(i, j) for i in range(-R, R + 1) for j in range(-R, R + 1)], np.int32)

Alu = mybir.AluOpType
Act = mybir.ActivationFunctionType
dt = mybir.dt

PLANES = ("du", "dv", "mm", "d1", "aa", "ad", "asx", "cr", "cg", "cb")

_BUILD_CACHE = {}


# --------------------------------------------------------------------------
# host prep: float32 mirror of the reference's per-gaussian projection math
# --------------------------------------------------------------------------

def _project_lanes(inputs):
    centers = inputs["centers"].astype(f32)
    scale = inputs["scale"].astype(f32)
    feat_dc = inputs["feat_dc"].astype(f32)
    opacity = inputs["opacity"].astype(f32)
    bgp = inputs["background_prob"].astype(f32)
    intr_all = inputs["camera_intrinsics"].astype(f32)
    c2w = inputs["camera_to_world"].astype(f32)
    first = inputs["first_ego_pose_world"].astype(f32)

    b, t = centers.shape[0], centers.shape[1]
    v = c2w.shape[2]
    cf = centers.reshape(b, t, -1, 3)
    sf = np.mean(scale, axis=-1).reshape(b, t, -1)
    colf = np.clip(feat_dc.reshape(b, t, -1, 3), 0.0, 1.0)
    op = np.clip(opacity[..., 0], 0.0, 1.0).reshape(b, t, -1)
    bg = bgp.reshape(b, t, -1)

    wh = np.concatenate([cf, np.ones(cf.shape[:-1] + (1,), f32)], axis=-1)
    world = np.einsum("bij,btnj->btni", first, wh).astype(f32)
    w2c = np.linalg.inv(c2w)

    lanes = []
    for bi in range(b):
        for ti in range(t):
            for vi in range(v):
                fx, fy, cx, cy = intr_all[bi, vi]
                cam = world[bi, ti] @ w2c[bi, ti, vi].T
                x, y, z = cam[:, 0], cam[:, 1], cam[:, 2]
                alpha = op[bi, ti]
                valid = (z > 1e-3) & np.all(np.isfinite(cam[:, :3]), axis=-1) & (alpha > 1e-5)
                zs = np.where(valid, z, f32(1.0))
                u = np.where(valid, x * fx / zs + cx, f32(-1e6))
                vv = np.where(valid, y * fy / zs + cy, f32(-1e6))
                sigma = np.clip((fx + fy) * f32(0.5) * np.abs(sf[bi, ti]) / np.maximum(zs, f32(1e-3)),
                                f32(0.75), f32(10.0))
                inb = (u >= -R - 1.0) & (u <= W + R) & (vv >= -R - 1.0) & (vv <= H + R)
                valid = valid & inb
                dyn = np.clip(f32(1.0) - bg[bi, ti], 0.0, 1.0)
                lanes.append(dict(u=u, v=vv, sigma=sigma, z=z, valid=valid,
                                  a_all=alpha, a_dyn=alpha * dyn,
                                  a_sta=alpha * (f32(1.0) - dyn),
                                  col=colf[bi, ti]))
    return lanes


def _build_pairs(lanes):
    """Per lane: keep (gaussian, offset) pairs, sorted by (pixel, z)."""
    segs = []           # entry arrays over all lanes
    pair_arrays = []
    for li, lane in enumerate(lanes):
        gi = np.nonzero(lane["valid"])[0]
        us, vs = lane["u"][gi], lane["v"][gi]
        x0 = np.floor(us).astype(np.int32)
        y0 = np.floor(vs).astype(np.int32)
        xo = x0[:, None] + _OFFS[None, :, 0]
        yo = y0[:, None] + _OFFS[None, :, 1]
        keep = (xo >= 0) & (xo < W) & (yo >= 0) & (yo < H)
        pg, po = np.nonzero(keep)
        g = gi[pg]
        pix = yo[pg, po].astype(np.int64) * W + xo[pg, po]
        order = np.lexsort((lane["z"][g], pix))
        g, pix = g[order], pix[order]
        xof = xo[pg, po][order].astype(f32)
        yof = yo[pg, po][order].astype(f32)
        sig = lane["sigma"][g]
        du = (lane["u"][g] - xof) / sig
        dv = (lane["v"][g] - yof) / sig
        col = lane["col"][g]
        pair_arrays.append(dict(
            du=du, dv=dv,
            aa=lane["a_all"][g], ad=lane["a_dyn"][g], asx=lane["a_sta"][g],
            cr=col[:, 0], cg=col[:, 1], cb=col[:, 2],
        ))
        seg_start = np.flatnonzero(np.r_[True, pix[1:] != pix[:-1]])
        seg_len = np.diff(np.r_[seg_start, len(pix)])
        segs.append(dict(lane=li, pix=pix[seg_start], start=seg_start, occ=seg_len))
    return segs, pair_arrays


def _plan_layout(segs):
    """Assign pixel entries to (core, class, partition, slot)."""
    occ_all = np.concatenate([s["occ"] for s in segs])
    occ_max = int(occ_all.max()) if len(occ_all) else 1
    classes = [c for c in (1, 2, 3, 4, 6, 8, 16) if c <= max(occ_max, 1)]
    while classes[-1] < occ_max:
        classes.append(classes[-1] * 2)

    lane_id = np.concatenate([np.full(len(s["occ"]), s["lane"], np.int32) for s in segs])
    pix_id = np.concatenate([s["pix"] for s in segs])
    # global pair start index (offset pair arrays per lane)
    lane_pair_off = np.cumsum([0] + [len(s["start"]) and 0 for s in segs])  # placeholder
    offs, acc = [], 0
    for s, _ in zip(segs, range(len(segs))):
        offs.append(acc)
        acc += int(s["start"][-1] + s["occ"][-1]) if len(s["start"]) else 0
    pair_start = np.concatenate([s["start"] + o for s, o in zip(segs, offs)])

    cls_of = np.searchsorted(np.array(classes), occ_all)
    plan = []
    col_base, out_base = 0, 0
    for ci, M in enumerate(classes):
        idx = np.nonzero(cls_of == ci)[0]
        ncls = len(idx)
        per_core = -(-ncls // NCORES) if ncls else 0
        npp = -(-per_core // NPART) if per_core else 0
        if npp == 0:
            plan.append(dict(M=M, npp=0, col_base=col_base, out_base=out_base,
                             entries=idx, core=None, p=None, k=None))
            continue
        j = np.arange(ncls)
        core = j % NCORES
        pos = j // NCORES
        p = pos % NPART
        k = pos // NPART
        plan.append(dict(M=M, npp=npp, col_base=col_base, out_base=out_base,
                         entries=idx, core=core, p=p, k=k))
        col_base += npp * (M + 1)
        out_base += npp
    F = col_base
    NPIX = out_base
    return dict(plan=plan, F=F, NPIX=NPIX, classes=classes,
                lane_id=lane_id, pix_id=pix_id, occ=occ_all, pair_start=pair_start)


def _fill_planes(layout, pair_arrays):
    F, NPIX = layout["F"], layout["NPIX"]
    pairs = {k: np.concatenate([pa[k] for pa in pair_arrays]) for k in pair_arrays[0]}

    planes = {}
    zero_names = ("du", "dv", "aa", "ad", "asx", "cr", "cg", "cb")
    for nm in zero_names:
        planes[nm] = np.zeros((NCORES, NPART, F), f32)
    planes["mm"] = np.ones((NCORES, NPART, F), f32)
    planes["d1"] = np.full((NCORES, NPART, F), f32(0.001))

    out_lin = [[] for _ in range(NCORES)]   # slot linear index p*NPIX+col per core
    out_dst = [[] for _ in range(NCORES)]   # lane*HW + pix per core

    for info in layout["plan"]:
        M, npp = info["M"], info["npp"]
        if npp == 0:
            continue
        cb = info["col_base"]
        # boundary columns for every (real or dummy) slot of this class
        planes["mm"][:, :, cb: cb + npp * (M + 1): (M + 1)] = 0.0
        planes["d1"][:, :, cb: cb + npp * (M + 1): (M + 1)] = 1.0

        e = info["entries"]
        core, p, k = info["core"], info["p"], info["k"]
        occ = layout["occ"][e]
        start = layout["pair_start"][e]
        bcol = cb + k * (M + 1)
        # per-pair flat positions
        tot = int(occ.sum())
        rep = np.repeat(np.arange(len(e)), occ)
        within = np.arange(tot) - np.repeat(np.cumsum(occ) - occ, occ)
        pcols = bcol[rep] + 1 + within
        pflat = (core[rep].astype(np.int64) * NPART + p[rep]) * F + pcols
        psrc = np.repeat(start, occ) + within
        for nm in zero_names:
            planes[nm].reshape(-1)[pflat] = pairs[nm][psrc]
        # output scatter map
        ocol = info["out_base"] + k
        for c in range(NCORES):
            msk = core == c
            out_lin[c].append((p[msk].astype(np.int64) * NPIX + ocol[msk]))
            out_dst[c].append(layout["lane_id"][e][msk].astype(np.int64) * HW
                              + layout["pix_id"][e][msk])
    out_lin = [np.concatenate(x) if x else np.zeros(0, np.int64) for x in out_lin]
    out_dst = [np.concatenate(x) if x else np.zeros(0, np.int64) for x in out_dst]
    return planes, out_lin, out_dst


def _sm_tables(lanes):
    sg = np.zeros((NPART, NLANES * SM_COLS), f32)
    vd = np.zeros((NPART, NLANES * SM_COLS), f32)
    for li, lane in enumerate(lanes):
        s = np.where(lane["valid"], lane["sigma"], f32(0.0)).reshape(NPART, SM_COLS)
        v = lane["valid"].astype(f32).reshape(NPART, SM_COLS)
        sg[:, li * SM_COLS:(li + 1) * SM_COLS] = s
        vd[:, li * SM_COLS:(li + 1) * SM_COLS] = v
    return sg, vd


# --------------------------------------------------------------------------
# device kernel
# --------------------------------------------------------------------------

def _build_bass(F, NPIX, class_meta):
    """class_meta: tuple of (M, npp, col_base, out_base).

    One input tensor pin [128, 10F] (plane order PLANES), one output tensor
    pout [128, 12*NPIX+16]: cols [0,9NPIX) = rgb for (branch,chan) c=bi*3+chi
    interleaved as (c, pix); [9NPIX,12NPIX) = aden per branch; last 16 = the
    (smv, trp) scalars in row 0.
    """
    nc = bacc.Bacc("TRN2", target_bir_lowering=False, debug=False,
                   num_devices=NCORES)
    P = {nm: i for i, nm in enumerate(PLANES)}
    pin = nc.declare_dram_parameter("pin", [NPART, 10 * F], dt.float32, isOutput=False)
    sg_in = nc.declare_dram_parameter("sg", [NPART, NLANES * SM_COLS], dt.float32, isOutput=False)
    vd_in = nc.declare_dram_parameter("vd", [NPART, NLANES * SM_COLS], dt.float32, isOutput=False)
    OUTW = 12 * NPIX + 16
    pout = nc.declare_dram_parameter("pout", [NPART, OUTW], dt.float32, isOutput=True)

    with tile.TileContext(nc) as tc:
        with tc.tile_pool(name="p", bufs=1) as pool:
            # per-chunk tiles so Tile's dependency tracking lets compute
            # start as soon as its own chunk lands; chunks alternate HWDGE
            # (sync) / SWDGE (gpsimd) queues to stream concurrently.
            # plane order: du,dv | mm,d1,aa | ad,asx | cr,cg,cb
            chunks = [(0, 2, nc.sync), (2, 5, nc.gpsimd), (5, 7, nc.sync),
                      (7, 10, nc.gpsimd)]
            ctiles = []
            for lo, hi, eng in chunks:
                ct = pool.tile([NPART, (hi - lo) * F], dt.float32,
                               tag=f"tin{lo}", name=f"tin{lo}")
                eng.dma_start(ct[:], pin[:, lo * F:hi * F])
                ctiles.append((lo, hi, ct))

            def pl(nm):
                i = P[nm]
                for lo, hi, ct in ctiles:
                    if lo <= i < hi:
                        return ct[:, (i - lo) * F:(i - lo + 1) * F]
                raise KeyError(nm)
            sgt = pool.tile([NPART, NLANES * SM_COLS], dt.float32, tag="sg", name="sg")
            nc.gpsimd.dma_start(sgt[:], sg_in[:])
            vdt = pool.tile([NPART, NLANES * SM_COLS], dt.float32, tag="vd", name="vd")
            nc.gpsimd.dma_start(vdt[:], vd_in[:])

            sq0 = pool.tile([NPART, F], dt.float32, tag="sq0", name="sq0")
            nc.scalar.activation(sq0[:], pl("du"), Act.Square)
            sq1 = pool.tile([NPART, F], dt.float32, tag="sq1", name="sq1")
            nc.scalar.activation(sq1[:], pl("dv"), Act.Square)
            r2 = pool.tile([NPART, F], dt.float32, tag="r2", name="r2")
            nc.vector.tensor_add(r2[:], sq0[:], sq1[:])
            g = pool.tile([NPART, F], dt.float32, tag="g", name="g")
            nc.scalar.activation(g[:], r2[:], Act.Exp, scale=-0.5)

            prgb = pool.tile([NPART, 9 * NPIX], dt.float32, tag="prgb", name="prgb")
            paux = pool.tile([NPART, 3 * NPIX + 16], dt.float32, tag="paux", name="paux")
            wbig = pool.tile([NPART, 9 * F], dt.float32, tag="wbig", name="wbig")
            Tbig = pool.tile([NPART, 3 * (F + 1)], dt.float32, tag="Tbig", name="Tbig")
            la = pool.tile([NPART, F], dt.float32, tag="la", name="la")
            d0 = pool.tile([NPART, F], dt.float32, tag="d0", name="d0")
            co = pool.tile([NPART, F], dt.float32, tag="co", name="co")

            for bi, apl in ((0, "aa"), (1, "ad"), (2, "asx")):
                Tb = Tbig[:, bi * (F + 1):(bi + 1) * (F + 1)]
                nc.vector.tensor_mul(la[:], g[:], pl(apl))
                nc.vector.tensor_scalar_min(la[:], la[:], 0.999)
                nc.vector.scalar_tensor_tensor(d0[:], la[:], -1.0, pl("mm"),
                                               Alu.mult, Alu.add)
                nc.vector.memset(Tb[:, 0:1], 1.0)
                nc.vector.tensor_tensor_scan(Tb[:, 1:F + 1], d0[:], pl("d1"),
                                             1.0, Alu.mult, Alu.max)
                nc.vector.tensor_mul(co[:], la[:], Tb[:, 0:F])
                for chi, cpl in ((0, "cr"), (1, "cg"), (2, "cb")):
                    c = bi * 3 + chi
                    nc.gpsimd.tensor_mul(wbig[:, c * F:(c + 1) * F], co[:], pl(cpl))

            # rgb: one 4D reduce per class over all 9 (branch,chan) planes
            w3 = wbig[:].rearrange("p (c f) -> p c f", c=9)
            o3 = prgb[:].rearrange("p (c n) -> p c n", c=9)
            for (M, npp, cb, ob) in class_meta:
                if npp == 0:
                    continue
                w4 = w3[:, :, cb: cb + npp * (M + 1)].rearrange(
                    "p c (n m) -> p c n m", m=M + 1)
                nc.vector.tensor_reduce(o3[:, :, ob: ob + npp], w4,
                                        axis=mybir.AxisListType.X, op=Alu.add)
            nc.vector.tensor_scalar_min(prgb[:], prgb[:], 1.0)
            nc.sync.dma_start(pout[:, 0:9 * NPIX], prgb[:])

            # aden: one strided extract per class over the 3 branches
            T3 = Tbig[:].rearrange("p (b f) -> p b f", b=3)
            a3 = paux[:, 0:3 * NPIX].rearrange("p (b n) -> p b n", b=3)
            for (M, npp, cb, ob) in class_meta:
                if npp == 0:
                    continue
                tl = T3[:, :, cb + 1: cb + 1 + npp * (M + 1)].rearrange(
                    "p b (n m) -> p b n m", m=M + 1)[:, :, :, M: M + 1]
                nc.vector.tensor_scalar(a3[:, :, ob: ob + npp], tl,
                                        -1.0, 1.0, Alu.mult, Alu.add)

            # ---- scalars: sm (replicated) and per-core touch partial ----
            parts = pool.tile([NPART, 16], dt.float32, tag="parts", name="parts")
            nc.vector.memset(parts[:], 0.0)
            tch = pool.tile([NPART, NPIX], dt.float32, tag="tch", name="tch")
            nc.vector.tensor_scalar(tch[:], paux[:, 0:NPIX],
                                    1e-6, None, Alu.is_gt)
            nc.vector.tensor_reduce(parts[:, 12:13], tch[:],
                                    axis=mybir.AxisListType.X, op=Alu.add)
            nc.vector.tensor_reduce(
                parts[:, 0:6],
                sgt[:].rearrange("p (l n) -> p l n", n=SM_COLS),
                axis=mybir.AxisListType.X, op=Alu.add)
            nc.vector.tensor_reduce(
                parts[:, 6:12],
                vdt[:].rearrange("p (l n) -> p l n", n=SM_COLS),
                axis=mybir.AxisListType.X, op=Alu.add)
            red = pool.tile([1, 16], dt.float32, tag="red", name="red")
            nc.gpsimd.tensor_reduce(red[:], parts[:],
                                    axis=mybir.AxisListType.C, op=Alu.add)
            cnt = pool.tile([1, 6], dt.float32, tag="cnt", name="cnt")
            nc.vector.tensor_scalar_max(cnt[:], red[:, 6:12], 1.0)
            inv = pool.tile([1, 6], dt.float32, tag="inv", name="inv")
            nc.vector.reciprocal(inv[:], cnt[:])
            smv6 = pool.tile([1, 6], dt.float32, tag="smv6", name="smv6")
            nc.vector.tensor_mul(smv6[:], red[:, 0:6], inv[:])
            smv1 = pool.tile([1, 1], dt.float32, tag="smv1", name="smv1")
            nc.vector.tensor_reduce(smv1[:], smv6[:],
                                    axis=mybir.AxisListType.X, op=Alu.add)
            nc.vector.tensor_scalar_mul(smv1[:], smv1[:], 1.0 / 6.0)
            nc.vector.tensor_copy(paux[0:1, 3 * NPIX:3 * NPIX + 1], smv1[:])
            nc.vector.tensor_copy(paux[0:1, 3 * NPIX + 1:3 * NPIX + 2], red[:, 12:13])
            nc.gpsimd.dma_start(pout[:, 9 * NPIX:], paux[:])
    nc.compile()
    return nc


# --------------------------------------------------------------------------
# entry point
# --------------------------------------------------------------------------

def kernel(_trace=False, **inputs):
    lanes = _project_lanes(inputs)
    segs, pair_arrays = _build_pairs(lanes)
    layout = _plan_layout(segs)
    planes, out_lin, out_dst = _fill_planes(layout, pair_arrays)
    sg, vd = _sm_tables(lanes)

    F, NPIX = layout["F"], layout["NPIX"]
    class_meta = tuple((p["M"], p["npp"], p["col_base"], p["out_base"])
                       for p in layout["plan"])
    key = (F, NPIX, class_meta)
    if key not in _BUILD_CACHE:
        _BUILD_CACHE[key] = _build_bass(F, NPIX, class_meta)
    nc = _BUILD_CACHE[key]

    in_maps = []
    for c in range(NCORES):
        pin = np.concatenate([planes[nm][c] for nm in PLANES], axis=1)
        in_maps.append({"pin": np.ascontiguousarray(pin), "sg": sg, "vd": vd})

    res = run_bass_kernel_spmd(nc, in_maps, list(range(NCORES)), trace=_trace)
    pouts = [res.results[c]["pout"] for c in range(NCORES)]

    shapes = {"rgb": (B, T, V, 3, H, W), "a": (B, T, V, 1, H, W)}
    out = {}
    for bi, bn in ((0, "all"), (1, "dyn"), (2, "sta")):
        chans = []
        for chi in range(3):
            cidx = bi * 3 + chi
            img = np.zeros(NLANES * HW, f32)
            for c in range(NCORES):
                plane = pouts[c][:, cidx * NPIX:(cidx + 1) * NPIX]
                img[out_dst[c]] = plane.reshape(-1)[out_lin[c]]
            chans.append(img.reshape(NLANES, H, W))
        out[f"rgb_{bn}"] = np.stack(chans, axis=1).reshape(shapes["rgb"])
        img = np.zeros(NLANES * HW, f32)
        for c in range(NCORES):
            plane = pouts[c][:, (9 + bi) * NPIX:(10 + bi) * NPIX]
            img[out_dst[c]] = plane.reshape(-1)[out_lin[c]]
        out[f"a_{bn}"] = img.reshape(NLANES, 1, H, W).reshape(shapes["a"])

    sm = f32(pouts[0][0, 12 * NPIX])
    tr_total = sum(float(pouts[c][0, 12 * NPIX + 1]) for c in range(NCORES))
    tr = f32(tr_total / (NLANES * HW))

    ret = (out["rgb_sta"], out["rgb_dyn"], out["rgb_all"],
           out["a_sta"], out["a_dyn"], out["a_all"],
           np.asarray(inputs["sem_proj_2d"], f32), sm, tr)
    if _trace:
        return ret, res
    return ret


# revision 14
# speedup vs baseline: 1.0529x; 1.0529x over previous
"""Trainium2 Bass kernel for nn_GaussianMaskRenderer.

Strategy
--------
The reference splats N=6144 depth-sorted gaussians (5x5 footprint, radius
always 2) into 224x448 framebuffers for 6 render lanes (b*t*view) x 3 alpha
branches (all/dyn/sta) with sequential front-to-back compositing.

The sequential per-pixel recurrence has the closed form
    T_i = max(T_{i-1} * (1 - la_i), 0.001),   contrib_i = la_i * T_{i-1}
i.e. a mult/max prefix scan over each pixel's depth-ordered splat list -- an
exact match for the VectorEngine's tensor_tensor_scan(op0=mult, op1=max).

Host side (pure indexing + the reference's trivial per-gaussian projection):
build per-pixel depth-ordered (gaussian,offset) pair lists, group pixels into
occupancy classes, and pack every pair's (du, dv, alpha_all/dyn/sta, rgb)
into dense [128, F] planes per core, with a boundary slot (d0=0, d1=1) in
front of every pixel run so one chained scan per branch resets per pixel.

Device side (8 NeuronCores, data-parallel over pixel work): exp/compositing/
reductions as dense vector ops; per-class 3D-AP reduces produce per-pixel
rgb/alpha; GPSIMD partition-reduce produces the sm/tr scalars.
"""

import numpy as np

import concourse.bass as bass  # noqa: F401  (AP helpers)
import concourse.mybir as mybir
import concourse.tile as tile
from concourse import bacc
from concourse.bass_utils import run_bass_kernel_spmd

B, T, V, GH, GW = 1, 2, 3, 32, 64
H, W = 224, 448
HW = H * W
R = 2
NLANES = B * T * V           # 6 render lanes
N = V * GH * GW              # 6144 gaussians per (b,t) frame
NCORES = 8
NPART = 128
SM_COLS = N // NPART         # 48 gaussian columns per lane in the sm table
f32 = np.float32

_OFFS = np.array([(i, j) for i in range(-R, R + 1) for j in range(-R, R + 1)], np.int32)

Alu = mybir.AluOpType
Act = mybir.ActivationFunctionType
dt = mybir.dt

PLANES = ("du", "dv", "mm", "d1", "aa", "ad", "asx", "cr", "cg", "cb")

_BUILD_CACHE = {}


# --------------------------------------------------------------------------
# host prep: float32 mirror of the reference's per-gaussian projection math
# --------------------------------------------------------------------------

def _project_lanes(inputs):
    centers = inputs["centers"].astype(f32)
    scale = inputs["scale"].astype(f32)
    feat_dc = inputs["feat_dc"].astype(f32)
    opacity = inputs["opacity"].astype(f32)
    bgp = inputs["background_prob"].astype(f32)
    intr_all = inputs["camera_intrinsics"].astype(f32)
    c2w = inputs["camera_to_world"].astype(f32)
    first = inputs["first_ego_pose_world"].astype(f32)

    b, t = centers.shape[0], centers.shape[1]
    v = c2w.shape[2]
    cf = centers.reshape(b, t, -1, 3)
    sf = np.mean(scale, axis=-1).reshape(b, t, -1)
    colf = np.clip(feat_dc.reshape(b, t, -1, 3), 0.0, 1.0)
    op = np.clip(opacity[..., 0], 0.0, 1.0).reshape(b, t, -1)
    bg = bgp.reshape(b, t, -1)

    wh = np.concatenate([cf, np.ones(cf.shape[:-1] + (1,), f32)], axis=-1)
    world = np.einsum("bij,btnj->btni", first, wh).astype(f32)
    w2c = np.linalg.inv(c2w)

    lanes = []
    for bi in range(b):
        for ti in range(t):
            for vi in range(v):
                fx, fy, cx, cy = intr_all[bi, vi]
                cam = world[bi, ti] @ w2c[bi, ti, vi].T
                x, y, z = cam[:, 0], cam[:, 1], cam[:, 2]
                alpha = op[bi, ti]
                valid = (z > 1e-3) & np.all(np.isfinite(cam[:, :3]), axis=-1) & (alpha > 1e-5)
                zs = np.where(valid, z, f32(1.0))
                u = np.where(valid, x * fx / zs + cx, f32(-1e6))
                vv = np.where(valid, y * fy / zs + cy, f32(-1e6))
                sigma = np.clip((fx + fy) * f32(0.5) * np.abs(sf[bi, ti]) / np.maximum(zs, f32(1e-3)),
                                f32(0.75), f32(10.0))
                inb = (u >= -R - 1.0) & (u <= W + R) & (vv >= -R - 1.0) & (vv <= H + R)
                valid = valid & inb
                dyn = np.clip(f32(1.0) - bg[bi, ti], 0.0, 1.0)
                lanes.append(dict(u=u, v=vv, sigma=sigma, z=z, valid=valid,
                                  a_all=alpha, a_dyn=alpha * dyn,
                                  a_sta=alpha * (f32(1.0) - dyn),
                                  col=colf[bi, ti]))
    return lanes


def _build_pairs(lanes):
    """Per lane: keep (gaussian, offset) pairs, sorted by (pixel, z)."""
    segs = []           # entry arrays over all lanes
    pair_arrays = []
    for li, lane in enumerate(lanes):
        gi = np.nonzero(lane["valid"])[0]
        us, vs = lane["u"][gi], lane["v"][gi]
        x0 = np.floor(us).astype(np.int32)
        y0 = np.floor(vs).astype(np.int32)
        xo = x0[:, None] + _OFFS[None, :, 0]
        yo = y0[:, None] + _OFFS[None, :, 1]
        keep = (xo >= 0) & (xo < W) & (yo >= 0) & (yo < H)
        pg, po = np.nonzero(keep)
        g = gi[pg]
        pix = yo[pg, po].astype(np.int64) * W + xo[pg, po]
        order = np.lexsort((lane["z"][g], pix))
        g, pix = g[order], pix[order]
        xof = xo[pg, po][order].astype(f32)
        yof = yo[pg, po][order].astype(f32)
        sig = lane["sigma"][g]
        du = (lane["u"][g] - xof) / sig
        dv = (lane["v"][g] - yof) / sig
        col = lane["col"][g]
        pair_arrays.append(dict(
            du=du, dv=dv,
            aa=lane["a_all"][g], ad=lane["a_dyn"][g], asx=lane["a_sta"][g],
            cr=col[:, 0], cg=col[:, 1], cb=col[:, 2],
        ))
        seg_start = np.flatnonzero(np.r_[True, pix[1:] != pix[:-1]])
        seg_len = np.diff(np.r_[seg_start, len(pix)])
        segs.append(dict(lane=li, pix=pix[seg_start], start=seg_start, occ=seg_len))
    return segs, pair_arrays


def _plan_layout(segs):
    """Assign pixel entries to (core, class, partition, slot)."""
    occ_all = np.concatenate([s["occ"] for s in segs])
    occ_max = int(occ_all.max()) if len(occ_all) else 1
    classes = [c for c in (1, 2, 3, 4, 6, 8, 16) if c <= max(occ_max, 1)]
    while classes[-1] < occ_max:
        classes.append(classes[-1] * 2)

    lane_id = np.concatenate([np.full(len(s["occ"]), s["lane"], np.int32) for s in segs])
    pix_id = np.concatenate([s["pix"] for s in segs])
    # global pair start index (offset pair arrays per lane)
    lane_pair_off = np.cumsum([0] + [len(s["start"]) and 0 for s in segs])  # placeholder
    offs, acc = [], 0
    for s, _ in zip(segs, range(len(segs))):
        offs.append(acc)
        acc += int(s["start"][-1] + s["occ"][-1]) if len(s["start"]) else 0
    pair_start = np.concatenate([s["start"] + o for s, o in zip(segs, offs)])

    cls_of = np.searchsorted(np.array(classes), occ_all)
    plan = []
    col_base, out_base = 0, 0
    for ci, M in enumerate(classes):
        idx = np.nonzero(cls_of == ci)[0]
        ncls = len(idx)
        per_core = -(-ncls // NCORES) if ncls else 0
        npp = -(-per_core // NPART) if per_core else 0
        if npp == 0:
            plan.append(dict(M=M, npp=0, col_base=col_base, out_base=out_base,
                             entries=idx, core=None, p=None, k=None))
            continue
        j = np.arange(ncls)
        core = j % NCORES
        pos = j // NCORES
        p = pos % NPART
        k = pos // NPART
        plan.append(dict(M=M, npp=npp, col_base=col_base, out_base=out_base,
                         entries=idx, core=core, p=p, k=k))
        col_base += npp * (M + 1)
        out_base += npp
    F = col_base
    NPIX = out_base
    return dict(plan=plan, F=F, NPIX=NPIX, classes=classes,
                lane_id=lane_id, pix_id=pix_id, occ=occ_all, pair_start=pair_start)


def _fill_planes(layout, pair_arrays):
    F, NPIX = layout["F"], layout["NPIX"]
    pairs = {k: np.concatenate([pa[k] for pa in pair_arrays]) for k in pair_arrays[0]}

    planes = {}
    zero_names = ("du", "dv", "aa", "ad", "asx", "cr", "cg", "cb")
    for nm in zero_names:
        planes[nm] = np.zeros((NCORES, NPART, F), f32)
    planes["mm"] = np.ones((NCORES, NPART, F), f32)
    planes["d1"] = np.full((NCORES, NPART, F), f32(0.001))

    out_lin = [[] for _ in range(NCORES)]   # slot linear index p*NPIX+col per core
    out_dst = [[] for _ in range(NCORES)]   # lane*HW + pix per core

    for info in layout["plan"]:
        M, npp = info["M"], info["npp"]
        if npp == 0:
            continue
        cb = info["col_base"]
        # boundary columns for every (real or dummy) slot of this class
        planes["mm"][:, :, cb: cb + npp * (M + 1): (M + 1)] = 0.0
        planes["d1"][:, :, cb: cb + npp * (M + 1): (M + 1)] = 1.0

        e = info["entries"]
        core, p, k = info["core"], info["p"], info["k"]
        occ = layout["occ"][e]
        start = layout["pair_start"][e]
        bcol = cb + k * (M + 1)
        # per-pair flat positions
        tot = int(occ.sum())
        rep = np.repeat(np.arange(len(e)), occ)
        within = np.arange(tot) - np.repeat(np.cumsum(occ) - occ, occ)
        pcols = bcol[rep] + 1 + within
        pflat = (core[rep].astype(np.int64) * NPART + p[rep]) * F + pcols
        psrc = np.repeat(start, occ) + within
        for nm in zero_names:
            planes[nm].reshape(-1)[pflat] = pairs[nm][psrc]
        # output scatter map
        ocol = info["out_base"] + k
        for c in range(NCORES):
            msk = core == c
            out_lin[c].append((p[msk].astype(np.int64) * NPIX + ocol[msk]))
            out_dst[c].append(layout["lane_id"][e][msk].astype(np.int64) * HW
                              + layout["pix_id"][e][msk])
    out_lin = [np.concatenate(x) if x else np.zeros(0, np.int64) for x in out_lin]
    out_dst = [np.concatenate(x) if x else np.zeros(0, np.int64) for x in out_dst]
    return planes, out_lin, out_dst


def _sm_tables(lanes):
    sg = np.zeros((NPART, NLANES * SM_COLS), f32)
    vd = np.zeros((NPART, NLANES * SM_COLS), f32)
    for li, lane in enumerate(lanes):
        s = np.where(lane["valid"], lane["sigma"], f32(0.0)).reshape(NPART, SM_COLS)
        v = lane["valid"].astype(f32).reshape(NPART, SM_COLS)
        sg[:, li * SM_COLS:(li + 1) * SM_COLS] = s
        vd[:, li * SM_COLS:(li + 1) * SM_COLS] = v
    return sg, vd


# --------------------------------------------------------------------------
# device kernel
# --------------------------------------------------------------------------

def _build_bass(F, NPIX, class_meta):
    """class_meta: tuple of (M, npp, col_base, out_base).

    One input tensor pin [128, 10F] (plane order PLANES), one output tensor
    pout [128, 12*NPIX+16]: cols [0,9NPIX) = rgb for (branch,chan) c=bi*3+chi
    interleaved as (c, pix); [9NPIX,12NPIX) = aden per branch; last 16 = the
    (smv, trp) scalars in row 0.
    """
    nc = bacc.Bacc("TRN2", target_bir_lowering=False, debug=False,
                   num_devices=NCORES)
    P = {nm: i for i, nm in enumerate(PLANES)}
    pin = nc.declare_dram_parameter("pin", [NPART, 10 * F], dt.float32, isOutput=False)
    sg_in = nc.declare_dram_parameter("sg", [NPART, NLANES * SM_COLS], dt.float32, isOutput=False)
    vd_in = nc.declare_dram_parameter("vd", [NPART, NLANES * SM_COLS], dt.float32, isOutput=False)
    OUTW = 12 * NPIX + 16
    pout = nc.declare_dram_parameter("pout", [NPART, OUTW], dt.float32, isOutput=True)

    with tile.TileContext(nc) as tc:
        with tc.tile_pool(name="p", bufs=1) as pool:
            # per-chunk tiles so Tile's dependency tracking lets compute
            # start as soon as its own chunk lands; chunks alternate HWDGE
            # (sync) / SWDGE (gpsimd) queues to stream concurrently.
            # plane order: du,dv | mm,d1,aa | ad,asx | cr,cg,cb
            chunks = [(0, 2, nc.sync), (2, 5, nc.gpsimd), (5, 7, nc.sync),
                      (7, 10, nc.gpsimd)]
            ctiles = []
            for lo, hi, eng in chunks:
                ct = pool.tile([NPART, (hi - lo) * F], dt.float32,
                               tag=f"tin{lo}", name=f"tin{lo}")
                eng.dma_start(ct[:], pin[:, lo * F:hi * F])
                ctiles.append((lo, hi, ct))

            def pl(nm):
                i = P[nm]
                for lo, hi, ct in ctiles:
                    if lo <= i < hi:
                        return ct[:, (i - lo) * F:(i - lo + 1) * F]
                raise KeyError(nm)
            sgt = pool.tile([NPART, NLANES * SM_COLS], dt.float32, tag="sg", name="sg")
            nc.gpsimd.dma_start(sgt[:], sg_in[:])
            vdt = pool.tile([NPART, NLANES * SM_COLS], dt.float32, tag="vd", name="vd")
            nc.gpsimd.dma_start(vdt[:], vd_in[:])

            sq0 = pool.tile([NPART, F], dt.float32, tag="sq0", name="sq0")
            nc.scalar.activation(sq0[:], pl("du"), Act.Square)
            sq1 = pool.tile([NPART, F], dt.float32, tag="sq1", name="sq1")
            nc.scalar.activation(sq1[:], pl("dv"), Act.Square)
            r2 = pool.tile([NPART, F], dt.float32, tag="r2", name="r2")
            nc.vector.tensor_add(r2[:], sq0[:], sq1[:])
            g = pool.tile([NPART, F], dt.float32, tag="g", name="g")
            nc.scalar.activation(g[:], r2[:], Act.Exp, scale=-0.5)

            prgb = pool.tile([NPART, 9 * NPIX], dt.float32, tag="prgb", name="prgb")
            paux = pool.tile([NPART, 3 * NPIX + 16], dt.float32, tag="paux", name="paux")
            wbig = pool.tile([NPART, 9 * F], dt.float32, tag="wbig", name="wbig")
            Tbig = pool.tile([NPART, 3 * (F + 1)], dt.float32, tag="Tbig", name="Tbig")
            la = pool.tile([NPART, F], dt.float32, tag="la", name="la")
            d0 = pool.tile([NPART, F], dt.float32, tag="d0", name="d0")
            co = pool.tile([NPART, F], dt.float32, tag="co", name="co")

            for bi, apl in ((0, "aa"), (1, "ad"), (2, "asx")):
                Tb = Tbig[:, bi * (F + 1):(bi + 1) * (F + 1)]
                nc.vector.tensor_mul(la[:], g[:], pl(apl))
                nc.vector.tensor_scalar_min(la[:], la[:], 0.999)
                nc.vector.scalar_tensor_tensor(d0[:], la[:], -1.0, pl("mm"),
                                               Alu.mult, Alu.add)
                nc.vector.memset(Tb[:, 0:1], 1.0)
                nc.vector.tensor_tensor_scan(Tb[:, 1:F + 1], d0[:], pl("d1"),
                                             1.0, Alu.mult, Alu.max)
                nc.vector.tensor_mul(co[:], la[:], Tb[:, 0:F])
                for chi, cpl in ((0, "cr"), (1, "cg"), (2, "cb")):
                    c = bi * 3 + chi
                    nc.gpsimd.tensor_mul(wbig[:, c * F:(c + 1) * F], co[:], pl(cpl))

            # rgb: per-branch 4D reduces (3 chans each) so each branch's
            # reduces start as soon as its own w-planes land
            for bi3 in range(3):
                w3 = wbig[:, bi3 * 3 * F:(bi3 + 1) * 3 * F].rearrange(
                    "p (c f) -> p c f", c=3)
                o3 = prgb[:, bi3 * 3 * NPIX:(bi3 + 1) * 3 * NPIX].rearrange(
                    "p (c n) -> p c n", c=3)
                for (M, npp, cb, ob) in class_meta:
                    if npp == 0:
                        continue
                    w4 = w3[:, :, cb: cb + npp * (M + 1)].rearrange(
                        "p c (n m) -> p c n m", m=M + 1)
                    nc.vector.tensor_reduce(o3[:, :, ob: ob + npp], w4,
                                            axis=mybir.AxisListType.X, op=Alu.add)
            nc.vector.tensor_scalar_min(prgb[:], prgb[:], 1.0)
            nc.sync.dma_start(pout[:, 0:9 * NPIX], prgb[:])

            # aden: one strided extract per class over the 3 branches
            T3 = Tbig[:].rearrange("p (b f) -> p b f", b=3)
            a3 = paux[:, 0:3 * NPIX].rearrange("p (b n) -> p b n", b=3)
            for (M, npp, cb, ob) in class_meta:
                if npp == 0:
                    continue
                tl = T3[:, :, cb + 1: cb + 1 + npp * (M + 1)].rearrange(
                    "p b (n m) -> p b n m", m=M + 1)[:, :, :, M: M + 1]
                nc.vector.tensor_scalar(a3[:, :, ob: ob + npp], tl,
                                        -1.0, 1.0, Alu.mult, Alu.add)

            # ---- scalars: sm (replicated) and per-core touch partial ----
            parts = pool.tile([NPART, 16], dt.float32, tag="parts", name="parts")
            nc.vector.memset(parts[:], 0.0)
            tch = pool.tile([NPART, NPIX], dt.float32, tag="tch", name="tch")
            nc.vector.tensor_scalar(tch[:], paux[:, 0:NPIX],
                                    1e-6, None, Alu.is_gt)
            nc.vector.tensor_reduce(parts[:, 12:13], tch[:],
                                    axis=mybir.AxisListType.X, op=Alu.add)
            nc.vector.tensor_reduce(
                parts[:, 0:6],
                sgt[:].rearrange("p (l n) -> p l n", n=SM_COLS),
                axis=mybir.AxisListType.X, op=Alu.add)
            nc.vector.tensor_reduce(
                parts[:, 6:12],
                vdt[:].rearrange("p (l n) -> p l n", n=SM_COLS),
                axis=mybir.AxisListType.X, op=Alu.add)
            red = pool.tile([1, 16], dt.float32, tag="red", name="red")
            nc.gpsimd.tensor_reduce(red[:], parts[:],
                                    axis=mybir.AxisListType.C, op=Alu.add)
            cnt = pool.tile([1, 6], dt.float32, tag="cnt", name="cnt")
            nc.vector.tensor_scalar_max(cnt[:], red[:, 6:12], 1.0)
            inv = pool.tile([1, 6], dt.float32, tag="inv", name="inv")
            nc.vector.reciprocal(inv[:], cnt[:])
            smv6 = pool.tile([1, 6], dt.float32, tag="smv6", name="smv6")
            nc.vector.tensor_mul(smv6[:], red[:, 0:6], inv[:])
            smv1 = pool.tile([1, 1], dt.float32, tag="smv1", name="smv1")
            nc.vector.tensor_reduce(smv1[:], smv6[:],
                                    axis=mybir.AxisListType.X, op=Alu.add)
            nc.vector.tensor_scalar_mul(smv1[:], smv1[:], 1.0 / 6.0)
            nc.vector.tensor_copy(paux[0:1, 3 * NPIX:3 * NPIX + 1], smv1[:])
            nc.vector.tensor_copy(paux[0:1, 3 * NPIX + 1:3 * NPIX + 2], red[:, 12:13])
            nc.gpsimd.dma_start(pout[:, 9 * NPIX:], paux[:])
    nc.compile()
    return nc


# --------------------------------------------------------------------------
# entry point
# --------------------------------------------------------------------------

def kernel(_trace=False, **inputs):
    lanes = _project_lanes(inputs)
    segs, pair_arrays = _build_pairs(lanes)
    layout = _plan_layout(segs)
    planes, out_lin, out_dst = _fill_planes(layout, pair_arrays)
    sg, vd = _sm_tables(lanes)

    F, NPIX = layout["F"], layout["NPIX"]
    class_meta = tuple((p["M"], p["npp"], p["col_base"], p["out_base"])
                       for p in layout["plan"])
    key = (F, NPIX, class_meta)
    if key not in _BUILD_CACHE:
        _BUILD_CACHE[key] = _build_bass(F, NPIX, class_meta)
    nc = _BUILD_CACHE[key]

    in_maps = []
    for c in range(NCORES):
        pin = np.concatenate([planes[nm][c] for nm in PLANES], axis=1)
        in_maps.append({"pin": np.ascontiguousarray(pin), "sg": sg, "vd": vd})

    res = run_bass_kernel_spmd(nc, in_maps, list(range(NCORES)), trace=_trace)
    pouts = [res.results[c]["pout"] for c in range(NCORES)]

    shapes = {"rgb": (B, T, V, 3, H, W), "a": (B, T, V, 1, H, W)}
    out = {}
    for bi, bn in ((0, "all"), (1, "dyn"), (2, "sta")):
        chans = []
        for chi in range(3):
            cidx = bi * 3 + chi
            img = np.zeros(NLANES * HW, f32)
            for c in range(NCORES):
                plane = pouts[c][:, cidx * NPIX:(cidx + 1) * NPIX]
                img[out_dst[c]] = plane.reshape(-1)[out_lin[c]]
            chans.append(img.reshape(NLANES, H, W))
        out[f"rgb_{bn}"] = np.stack(chans, axis=1).reshape(shapes["rgb"])
        img = np.zeros(NLANES * HW, f32)
        for c in range(NCORES):
            plane = pouts[c][:, (9 + bi) * NPIX:(10 + bi) * NPIX]
            img[out_dst[c]] = plane.reshape(-1)[out_lin[c]]
        out[f"a_{bn}"] = img.reshape(NLANES, 1, H, W).reshape(shapes["a"])

    sm = f32(pouts[0][0, 12 * NPIX])
    tr_total = sum(float(pouts[c][0, 12 * NPIX + 1]) for c in range(NCORES))
    tr = f32(tr_total / (NLANES * HW))

    ret = (out["rgb_sta"], out["rgb_dyn"], out["rgb_all"],
           out["a_sta"], out["a_dyn"], out["a_all"],
           np.asarray(inputs["sem_proj_2d"], f32), sm, tr)
    if _trace:
        return ret, res
    return ret


# revision 15
# speedup vs baseline: 1.0556x; 1.0026x over previous
"""Trainium2 Bass kernel for nn_GaussianMaskRenderer.

Strategy
--------
The reference splats N=6144 depth-sorted gaussians (5x5 footprint, radius
always 2) into 224x448 framebuffers for 6 render lanes (b*t*view) x 3 alpha
branches (all/dyn/sta) with sequential front-to-back compositing.

The sequential per-pixel recurrence has the closed form
    T_i = max(T_{i-1} * (1 - la_i), 0.001),   contrib_i = la_i * T_{i-1}
i.e. a mult/max prefix scan over each pixel's depth-ordered splat list -- an
exact match for the VectorEngine's tensor_tensor_scan(op0=mult, op1=max).

Host side (pure indexing + the reference's trivial per-gaussian projection):
build per-pixel depth-ordered (gaussian,offset) pair lists, group pixels into
occupancy classes, and pack every pair's (du, dv, alpha_all/dyn/sta, rgb)
into dense [128, F] planes per core, with a boundary slot (d0=0, d1=1) in
front of every pixel run so one chained scan per branch resets per pixel.

Device side (8 NeuronCores, data-parallel over pixel work): exp/compositing/
reductions as dense vector ops; per-class 3D-AP reduces produce per-pixel
rgb/alpha; GPSIMD partition-reduce produces the sm/tr scalars.
"""

import numpy as np

import concourse.bass as bass  # noqa: F401  (AP helpers)
import concourse.mybir as mybir
import concourse.tile as tile
from concourse import bacc
from concourse.bass_utils import run_bass_kernel_spmd

B, T, V, GH, GW = 1, 2, 3, 32, 64
H, W = 224, 448
HW = H * W
R = 2
NLANES = B * T * V           # 6 render lanes
N = V * GH * GW              # 6144 gaussians per (b,t) frame
NCORES = 8
NPART = 128
SM_COLS = N // NPART         # 48 gaussian columns per lane in the sm table
f32 = np.float32

_OFFS = np.array([(i, j) for i in range(-R, R + 1) for j in range(-R, R + 1)], np.int32)

Alu = mybir.AluOpType
Act = mybir.ActivationFunctionType
dt = mybir.dt

PLANES = ("du", "dv", "mm", "d1", "aa", "ad", "asx", "cr", "cg", "cb")

_BUILD_CACHE = {}


# --------------------------------------------------------------------------
# host prep: float32 mirror of the reference's per-gaussian projection math
# --------------------------------------------------------------------------

def _project_lanes(inputs):
    centers = inputs["centers"].astype(f32)
    scale = inputs["scale"].astype(f32)
    feat_dc = inputs["feat_dc"].astype(f32)
    opacity = inputs["opacity"].astype(f32)
    bgp = inputs["background_prob"].astype(f32)
    intr_all = inputs["camera_intrinsics"].astype(f32)
    c2w = inputs["camera_to_world"].astype(f32)
    first = inputs["first_ego_pose_world"].astype(f32)

    b, t = centers.shape[0], centers.shape[1]
    v = c2w.shape[2]
    cf = centers.reshape(b, t, -1, 3)
    sf = np.mean(scale, axis=-1).reshape(b, t, -1)
    colf = np.clip(feat_dc.reshape(b, t, -1, 3), 0.0, 1.0)
    op = np.clip(opacity[..., 0], 0.0, 1.0).reshape(b, t, -1)
    bg = bgp.reshape(b, t, -1)

    wh = np.concatenate([cf, np.ones(cf.shape[:-1] + (1,), f32)], axis=-1)
    world = np.einsum("bij,btnj->btni", first, wh).astype(f32)
    w2c = np.linalg.inv(c2w)

    lanes = []
    for bi in range(b):
        for ti in range(t):
            for vi in range(v):
                fx, fy, cx, cy = intr_all[bi, vi]
                cam = world[bi, ti] @ w2c[bi, ti, vi].T
                x, y, z = cam[:, 0], cam[:, 1], cam[:, 2]
                alpha = op[bi, ti]
                valid = (z > 1e-3) & np.all(np.isfinite(cam[:, :3]), axis=-1) & (alpha > 1e-5)
                zs = np.where(valid, z, f32(1.0))
                u = np.where(valid, x * fx / zs + cx, f32(-1e6))
                vv = np.where(valid, y * fy / zs + cy, f32(-1e6))
                sigma = np.clip((fx + fy) * f32(0.5) * np.abs(sf[bi, ti]) / np.maximum(zs, f32(1e-3)),
                                f32(0.75), f32(10.0))
                inb = (u >= -R - 1.0) & (u <= W + R) & (vv >= -R - 1.0) & (vv <= H + R)
                valid = valid & inb
                dyn = np.clip(f32(1.0) - bg[bi, ti], 0.0, 1.0)
                lanes.append(dict(u=u, v=vv, sigma=sigma, z=z, valid=valid,
                                  a_all=alpha, a_dyn=alpha * dyn,
                                  a_sta=alpha * (f32(1.0) - dyn),
                                  col=colf[bi, ti]))
    return lanes


def _build_pairs(lanes):
    """Per lane: keep (gaussian, offset) pairs, sorted by (pixel, z)."""
    segs = []           # entry arrays over all lanes
    pair_arrays = []
    for li, lane in enumerate(lanes):
        gi = np.nonzero(lane["valid"])[0]
        us, vs = lane["u"][gi], lane["v"][gi]
        x0 = np.floor(us).astype(np.int32)
        y0 = np.floor(vs).astype(np.int32)
        xo = x0[:, None] + _OFFS[None, :, 0]
        yo = y0[:, None] + _OFFS[None, :, 1]
        keep = (xo >= 0) & (xo < W) & (yo >= 0) & (yo < H)
        pg, po = np.nonzero(keep)
        g = gi[pg]
        pix = yo[pg, po].astype(np.int64) * W + xo[pg, po]
        order = np.lexsort((lane["z"][g], pix))
        g, pix = g[order], pix[order]
        xof = xo[pg, po][order].astype(f32)
        yof = yo[pg, po][order].astype(f32)
        sig = lane["sigma"][g]
        du = (lane["u"][g] - xof) / sig
        dv = (lane["v"][g] - yof) / sig
        col = lane["col"][g]
        pair_arrays.append(dict(
            du=du, dv=dv,
            aa=lane["a_all"][g], ad=lane["a_dyn"][g], asx=lane["a_sta"][g],
            cr=col[:, 0], cg=col[:, 1], cb=col[:, 2],
        ))
        seg_start = np.flatnonzero(np.r_[True, pix[1:] != pix[:-1]])
        seg_len = np.diff(np.r_[seg_start, len(pix)])
        segs.append(dict(lane=li, pix=pix[seg_start], start=seg_start, occ=seg_len))
    return segs, pair_arrays


def _plan_layout(segs):
    """Assign pixel entries to (core, class, partition, slot)."""
    occ_all = np.concatenate([s["occ"] for s in segs])
    occ_max = int(occ_all.max()) if len(occ_all) else 1
    classes = [c for c in (1, 2, 3, 4, 6, 8, 16) if c <= max(occ_max, 1)]
    while classes[-1] < occ_max:
        classes.append(classes[-1] * 2)

    lane_id = np.concatenate([np.full(len(s["occ"]), s["lane"], np.int32) for s in segs])
    pix_id = np.concatenate([s["pix"] for s in segs])
    # global pair start index (offset pair arrays per lane)
    lane_pair_off = np.cumsum([0] + [len(s["start"]) and 0 for s in segs])  # placeholder
    offs, acc = [], 0
    for s, _ in zip(segs, range(len(segs))):
        offs.append(acc)
        acc += int(s["start"][-1] + s["occ"][-1]) if len(s["start"]) else 0
    pair_start = np.concatenate([s["start"] + o for s, o in zip(segs, offs)])

    cls_of = np.searchsorted(np.array(classes), occ_all)
    plan = []
    col_base, out_base = 0, 0
    for ci, M in enumerate(classes):
        idx = np.nonzero(cls_of == ci)[0]
        ncls = len(idx)
        per_core = -(-ncls // NCORES) if ncls else 0
        npp = -(-per_core // NPART) if per_core else 0
        if npp == 0:
            plan.append(dict(M=M, npp=0, col_base=col_base, out_base=out_base,
                             entries=idx, core=None, p=None, k=None))
            continue
        j = np.arange(ncls)
        core = j % NCORES
        pos = j // NCORES
        p = pos % NPART
        k = pos // NPART
        plan.append(dict(M=M, npp=npp, col_base=col_base, out_base=out_base,
                         entries=idx, core=core, p=p, k=k))
        col_base += npp * (M + 1)
        out_base += npp
    F = col_base
    NPIX = out_base
    return dict(plan=plan, F=F, NPIX=NPIX, classes=classes,
                lane_id=lane_id, pix_id=pix_id, occ=occ_all, pair_start=pair_start)


def _fill_planes(layout, pair_arrays):
    F, NPIX = layout["F"], layout["NPIX"]
    pairs = {k: np.concatenate([pa[k] for pa in pair_arrays]) for k in pair_arrays[0]}

    planes = {}
    zero_names = ("du", "dv", "aa", "ad", "asx", "cr", "cg", "cb")
    for nm in zero_names:
        planes[nm] = np.zeros((NCORES, NPART, F), f32)
    planes["mm"] = np.ones((NCORES, NPART, F), f32)
    planes["d1"] = np.full((NCORES, NPART, F), f32(0.001))

    out_lin = [[] for _ in range(NCORES)]   # slot linear index p*NPIX+col per core
    out_dst = [[] for _ in range(NCORES)]   # lane*HW + pix per core

    for info in layout["plan"]:
        M, npp = info["M"], info["npp"]
        if npp == 0:
            continue
        cb = info["col_base"]
        # boundary columns for every (real or dummy) slot of this class
        planes["mm"][:, :, cb: cb + npp * (M + 1): (M + 1)] = 0.0
        planes["d1"][:, :, cb: cb + npp * (M + 1): (M + 1)] = 1.0

        e = info["entries"]
        core, p, k = info["core"], info["p"], info["k"]
        occ = layout["occ"][e]
        start = layout["pair_start"][e]
        bcol = cb + k * (M + 1)
        # per-pair flat positions
        tot = int(occ.sum())
        rep = np.repeat(np.arange(len(e)), occ)
        within = np.arange(tot) - np.repeat(np.cumsum(occ) - occ, occ)
        pcols = bcol[rep] + 1 + within
        pflat = (core[rep].astype(np.int64) * NPART + p[rep]) * F + pcols
        psrc = np.repeat(start, occ) + within
        for nm in zero_names:
            planes[nm].reshape(-1)[pflat] = pairs[nm][psrc]
        # output scatter map
        ocol = info["out_base"] + k
        for c in range(NCORES):
            msk = core == c
            out_lin[c].append((p[msk].astype(np.int64) * NPIX + ocol[msk]))
            out_dst[c].append(layout["lane_id"][e][msk].astype(np.int64) * HW
                              + layout["pix_id"][e][msk])
    out_lin = [np.concatenate(x) if x else np.zeros(0, np.int64) for x in out_lin]
    out_dst = [np.concatenate(x) if x else np.zeros(0, np.int64) for x in out_dst]
    return planes, out_lin, out_dst


def _sm_tables(lanes):
    sg = np.zeros((NPART, NLANES * SM_COLS), f32)
    vd = np.zeros((NPART, NLANES * SM_COLS), f32)
    for li, lane in enumerate(lanes):
        s = np.where(lane["valid"], lane["sigma"], f32(0.0)).reshape(NPART, SM_COLS)
        v = lane["valid"].astype(f32).reshape(NPART, SM_COLS)
        sg[:, li * SM_COLS:(li + 1) * SM_COLS] = s
        vd[:, li * SM_COLS:(li + 1) * SM_COLS] = v
    return sg, vd


# --------------------------------------------------------------------------
# device kernel
# --------------------------------------------------------------------------

def _build_bass(F, NPIX, class_meta):
    """class_meta: tuple of (M, npp, col_base, out_base).

    One input tensor pin [128, 10F] (plane order PLANES), one output tensor
    pout [128, 12*NPIX+16]: cols [0,9NPIX) = rgb for (branch,chan) c=bi*3+chi
    interleaved as (c, pix); [9NPIX,12NPIX) = aden per branch; last 16 = the
    (smv, trp) scalars in row 0.
    """
    nc = bacc.Bacc("TRN2", target_bir_lowering=False, debug=False,
                   num_devices=NCORES)
    P = {nm: i for i, nm in enumerate(PLANES)}
    pin = nc.declare_dram_parameter("pin", [NPART, 10 * F], dt.float32, isOutput=False)
    sg_in = nc.declare_dram_parameter("sg", [NPART, NLANES * SM_COLS], dt.float32, isOutput=False)
    vd_in = nc.declare_dram_parameter("vd", [NPART, NLANES * SM_COLS], dt.float32, isOutput=False)
    OUTW = 12 * NPIX + 16
    pout = nc.declare_dram_parameter("pout", [NPART, OUTW], dt.float32, isOutput=True)

    with tile.TileContext(nc) as tc:
        with tc.tile_pool(name="p", bufs=1) as pool:
            # per-chunk tiles so Tile's dependency tracking lets compute
            # start as soon as its own chunk lands; chunks alternate HWDGE
            # (sync) / SWDGE (gpsimd) queues to stream concurrently.
            # plane order: du,dv | mm,d1,aa | ad,asx | cr,cg,cb
            chunks = [(0, 2, nc.sync), (2, 5, nc.gpsimd), (5, 7, nc.sync),
                      (7, 10, nc.gpsimd)]
            ctiles = []
            for lo, hi, eng in chunks:
                ct = pool.tile([NPART, (hi - lo) * F], dt.float32,
                               tag=f"tin{lo}", name=f"tin{lo}")
                eng.dma_start(ct[:], pin[:, lo * F:hi * F])
                ctiles.append((lo, hi, ct))

            def pl(nm):
                i = P[nm]
                for lo, hi, ct in ctiles:
                    if lo <= i < hi:
                        return ct[:, (i - lo) * F:(i - lo + 1) * F]
                raise KeyError(nm)
            sgt = pool.tile([NPART, NLANES * SM_COLS], dt.float32, tag="sg", name="sg")
            nc.gpsimd.dma_start(sgt[:], sg_in[:])
            vdt = pool.tile([NPART, NLANES * SM_COLS], dt.float32, tag="vd", name="vd")
            nc.gpsimd.dma_start(vdt[:], vd_in[:])

            sq0 = pool.tile([NPART, F], dt.float32, tag="sq0", name="sq0")
            nc.scalar.activation(sq0[:], pl("du"), Act.Square)
            sq1 = pool.tile([NPART, F], dt.float32, tag="sq1", name="sq1")
            nc.scalar.activation(sq1[:], pl("dv"), Act.Square)
            r2 = pool.tile([NPART, F], dt.float32, tag="r2", name="r2")
            nc.vector.tensor_add(r2[:], sq0[:], sq1[:])
            g = pool.tile([NPART, F], dt.float32, tag="g", name="g")
            nc.scalar.activation(g[:], r2[:], Act.Exp, scale=-0.5)

            prgb = pool.tile([NPART, 9 * NPIX], dt.float32, tag="prgb", name="prgb")
            paux = pool.tile([NPART, 3 * NPIX + 16], dt.float32, tag="paux", name="paux")
            wbig = pool.tile([NPART, 9 * F], dt.float32, tag="wbig", name="wbig")
            Tbig = pool.tile([NPART, 3 * (F + 1)], dt.float32, tag="Tbig", name="Tbig")
            la = pool.tile([NPART, F], dt.float32, tag="la", name="la")
            d0 = pool.tile([NPART, F], dt.float32, tag="d0", name="d0")
            co = pool.tile([NPART, F], dt.float32, tag="co", name="co")

            for bi, apl in ((0, "aa"), (1, "ad"), (2, "asx")):
                Tb = Tbig[:, bi * (F + 1):(bi + 1) * (F + 1)]
                nc.vector.tensor_mul(la[:], g[:], pl(apl))
                nc.vector.tensor_scalar_min(la[:], la[:], 0.999)
                nc.vector.scalar_tensor_tensor(d0[:], la[:], -1.0, pl("mm"),
                                               Alu.mult, Alu.add)
                nc.vector.memset(Tb[:, 0:1], 1.0)
                nc.vector.tensor_tensor_scan(Tb[:, 1:F + 1], d0[:], pl("d1"),
                                             1.0, Alu.mult, Alu.max)
                nc.vector.tensor_mul(co[:], la[:], Tb[:, 0:F])
                for chi, cpl in ((0, "cr"), (1, "cg"), (2, "cb")):
                    c = bi * 3 + chi
                    nc.gpsimd.tensor_mul(wbig[:, c * F:(c + 1) * F], co[:], pl(cpl))

            # rgb: per-branch 4D reduces (3 chans each) so each branch's
            # reduces start as soon as its own w-planes land
            for bi3 in range(3):
                w3 = wbig[:, bi3 * 3 * F:(bi3 + 1) * 3 * F].rearrange(
                    "p (c f) -> p c f", c=3)
                o3 = prgb[:, bi3 * 3 * NPIX:(bi3 + 1) * 3 * NPIX].rearrange(
                    "p (c n) -> p c n", c=3)
                for (M, npp, cb, ob) in class_meta:
                    if npp == 0:
                        continue
                    w4 = w3[:, :, cb: cb + npp * (M + 1)].rearrange(
                        "p c (n m) -> p c n m", m=M + 1)
                    nc.vector.tensor_reduce(o3[:, :, ob: ob + npp], w4,
                                            axis=mybir.AxisListType.X, op=Alu.add)
            nc.vector.tensor_scalar_min(prgb[:], prgb[:], 1.0)
            nc.sync.dma_start(pout[:, 0:9 * NPIX], prgb[:])

            # aden: one strided extract per class over the 3 branches
            T3 = Tbig[:].rearrange("p (b f) -> p b f", b=3)
            a3 = paux[:, 0:3 * NPIX].rearrange("p (b n) -> p b n", b=3)
            for (M, npp, cb, ob) in class_meta:
                if npp == 0:
                    continue
                tl = T3[:, :, cb + 1: cb + 1 + npp * (M + 1)].rearrange(
                    "p b (n m) -> p b n m", m=M + 1)[:, :, :, M: M + 1]
                nc.vector.tensor_scalar(a3[:, :, ob: ob + npp], tl,
                                        -1.0, 1.0, Alu.mult, Alu.add)

            # ---- scalars: sm (replicated) and per-core touch partial ----
            parts = pool.tile([NPART, 16], dt.float32, tag="parts", name="parts")
            nc.vector.memset(parts[:], 0.0)
            tch = pool.tile([NPART, NPIX], dt.float32, tag="tch", name="tch")
            nc.vector.tensor_scalar(tch[:], paux[:, 0:NPIX],
                                    1e-6, None, Alu.is_gt)
            nc.vector.tensor_reduce(parts[:, 12:13], tch[:],
                                    axis=mybir.AxisListType.X, op=Alu.add)
            nc.vector.tensor_reduce(
                parts[:, 0:6],
                sgt[:].rearrange("p (l n) -> p l n", n=SM_COLS),
                axis=mybir.AxisListType.X, op=Alu.add)
            nc.vector.tensor_reduce(
                parts[:, 6:12],
                vdt[:].rearrange("p (l n) -> p l n", n=SM_COLS),
                axis=mybir.AxisListType.X, op=Alu.add)
            red = pool.tile([1, 16], dt.float32, tag="red", name="red")
            nc.gpsimd.tensor_reduce(red[:], parts[:],
                                    axis=mybir.AxisListType.C, op=Alu.add)
            cnt = pool.tile([1, 6], dt.float32, tag="cnt", name="cnt")
            nc.vector.tensor_scalar_max(cnt[:], red[:, 6:12], 1.0)
            inv = pool.tile([1, 6], dt.float32, tag="inv", name="inv")
            nc.vector.reciprocal(inv[:], cnt[:])
            smv6 = pool.tile([1, 6], dt.float32, tag="smv6", name="smv6")
            nc.vector.tensor_mul(smv6[:], red[:, 0:6], inv[:])
            smv1 = pool.tile([1, 1], dt.float32, tag="smv1", name="smv1")
            nc.vector.tensor_reduce(smv1[:], smv6[:],
                                    axis=mybir.AxisListType.X, op=Alu.add)
            nc.vector.tensor_scalar_mul(smv1[:], smv1[:], 1.0 / 6.0)
            nc.vector.tensor_copy(paux[0:1, 3 * NPIX:3 * NPIX + 1], smv1[:])
            nc.vector.tensor_copy(paux[0:1, 3 * NPIX + 1:3 * NPIX + 2], red[:, 12:13])
            nc.sync.dma_start(pout[:, 9 * NPIX:], paux[:])
    nc.compile()
    return nc


# --------------------------------------------------------------------------
# entry point
# --------------------------------------------------------------------------

def kernel(_trace=False, **inputs):
    lanes = _project_lanes(inputs)
    segs, pair_arrays = _build_pairs(lanes)
    layout = _plan_layout(segs)
    planes, out_lin, out_dst = _fill_planes(layout, pair_arrays)
    sg, vd = _sm_tables(lanes)

    F, NPIX = layout["F"], layout["NPIX"]
    class_meta = tuple((p["M"], p["npp"], p["col_base"], p["out_base"])
                       for p in layout["plan"])
    key = (F, NPIX, class_meta)
    if key not in _BUILD_CACHE:
        _BUILD_CACHE[key] = _build_bass(F, NPIX, class_meta)
    nc = _BUILD_CACHE[key]

    in_maps = []
    for c in range(NCORES):
        pin = np.concatenate([planes[nm][c] for nm in PLANES], axis=1)
        in_maps.append({"pin": np.ascontiguousarray(pin), "sg": sg, "vd": vd})

    res = run_bass_kernel_spmd(nc, in_maps, list(range(NCORES)), trace=_trace)
    pouts = [res.results[c]["pout"] for c in range(NCORES)]

    shapes = {"rgb": (B, T, V, 3, H, W), "a": (B, T, V, 1, H, W)}
    out = {}
    for bi, bn in ((0, "all"), (1, "dyn"), (2, "sta")):
        chans = []
        for chi in range(3):
            cidx = bi * 3 + chi
            img = np.zeros(NLANES * HW, f32)
            for c in range(NCORES):
                plane = pouts[c][:, cidx * NPIX:(cidx + 1) * NPIX]
                img[out_dst[c]] = plane.reshape(-1)[out_lin[c]]
            chans.append(img.reshape(NLANES, H, W))
        out[f"rgb_{bn}"] = np.stack(chans, axis=1).reshape(shapes["rgb"])
        img = np.zeros(NLANES * HW, f32)
        for c in range(NCORES):
            plane = pouts[c][:, (9 + bi) * NPIX:(10 + bi) * NPIX]
            img[out_dst[c]] = plane.reshape(-1)[out_lin[c]]
        out[f"a_{bn}"] = img.reshape(NLANES, 1, H, W).reshape(shapes["a"])

    sm = f32(pouts[0][0, 12 * NPIX])
    tr_total = sum(float(pouts[c][0, 12 * NPIX + 1]) for c in range(NCORES))
    tr = f32(tr_total / (NLANES * HW))

    ret = (out["rgb_sta"], out["rgb_dyn"], out["rgb_all"],
           out["a_sta"], out["a_dyn"], out["a_all"],
           np.asarray(inputs["sem_proj_2d"], f32), sm, tr)
    if _trace:
        return ret, res
    return ret
